# revision 29
# baseline (speedup 1.0000x reference)
"""Causal self-attention (B=2, T=2048, C=1024, H=16) on 8 trn2 NeuronCores.

Sharding: tensor-parallel over heads. Each core owns 2 heads (128 features):
  - qkv projection of the full sequence onto its 384 w_attn columns
  - causal attention for its 2 heads (both batches)
  - partial c_proj: y_local [4096,128] @ w_proj[rows] -> [4096,1024] partial
The 8 partial outputs are summed on the host (the "all-reduce after c_proj"),
plus b_proj.

Everything on-device runs in bf16 with fp32 PSUM accumulation (measured
end-to-end absmax-relative error vs the fp32 reference ~3e-3 in numpy
simulation). bf16 halves DMA traffic and DVE element-wise cost, and runs
matmuls at full rate for any moving width (so causal diagonal blocks trim
to 128-column granularity).

Layout: x is transposed on the host (x^T [1024, 4096], bf16) so the qkv
projection consumes it directly as the moving operand; Q^T/K^T come out in
[feature, token] layout feeding the S = K-stationary attention matmul, with
softmax denominators obtained from an extra ones-column in V. Normalization
uses reciprocal_approx_fast on the PSUM denominator row + gpsimd partition
broadcast (no DRAM round trip).
"""

import sys

sys.path.insert(0, "/opt/trn_rl_repo")

import numpy as np

N_CORES = 8
B, T, C = 2, 2048, 1024
H, D = 16, 64
HPC = H // N_CORES            # heads per core
F = HPC * D                   # local feature width = 128
BT = B * T                    # 4096 tokens
TCH = 512                     # token chunk (moving-operand width)
NCH = BT // TCH               # 8 token chunks
KB = 128                      # kv block size
NQC = T // TCH                # 4 query chunks per batch

_COMPILED = {}


def _build():
    import concourse.bass as bass
    import concourse.mybir as mybir
    import concourse.tile as tile
    from concourse import bacc

    f32, bf16 = mybir.dt.float32, mybir.dt.bfloat16
    Exp = mybir.ActivationFunctionType.Exp

    nc = bacc.Bacc("TRN2", target_bir_lowering=False, debug=False,
                   num_devices=N_CORES)

    xt = nc.dram_tensor("xt", [C // 128, 128, BT], bf16, kind="ExternalInput")
    wqkv = nc.dram_tensor("wqkv", [C // 128, 128, 3 * F], bf16,
                          kind="ExternalInput")
    bqkv = nc.dram_tensor("bqkv", [F, 3], f32, kind="ExternalInput")
    wp = nc.dram_tensor("wp", [F, C], bf16, kind="ExternalInput")
    tri = nc.dram_tensor("tri", [KB, KB], bf16, kind="ExternalInput")
    eye = nc.dram_tensor("eye", [128, 128], bf16, kind="ExternalInput")
    # sel33[0, p] = 1 for p < 64; sel33[32, p] = 1 for p >= 64; else 0.
    # One matmul (sel33^T @ rb2) broadcasts per-head reciprocal rows to a
    # [128, 512] tile: partitions 0-63 get rb2 row 0, 64-127 get row 32.
    sel = nc.dram_tensor("sel", [33, 128], bf16, kind="ExternalInput")
    out = nc.dram_tensor("out", [BT, C], bf16, kind="ExternalOutput")

    with tile.TileContext(nc) as tc, \
         nc.allow_low_precision(reason="bf16 matmul pipeline, fp32 psum"):
        with tc.tile_pool(name="const", bufs=1) as cpool, \
             tc.tile_pool(name="seq", bufs=1) as seq, \
             tc.tile_pool(name="work", bufs=4) as work, \
             tc.tile_pool(name="psBig", bufs=2, space="PSUM") as psBig, \
             tc.tile_pool(name="psS", bufs=2, space="PSUM") as psS, \
             tc.tile_pool(name="psY", bufs=2, space="PSUM") as psY:

            # ---- resident x (bf16, 64KB/partition) + constants ----
            # issue order matters: interleave x chunk 0 and w per-cb so the
            # first qkv matmuls can start as early as possible; the rest of
            # x follows as one big per-cb transfer. Small constants go on
            # the scalar (Activation) hwdge queue, which is idle at start.
            x_sb = cpool.tile([128, C // 128, BT], bf16)
            w_sb = cpool.tile([128, C // 128, 3 * F], bf16)
            for cb in range(8):
                nc.sync.dma_start(x_sb[:, cb, 0:TCH], xt[cb, :, 0:TCH])
                nc.sync.dma_start(w_sb[:, cb, :], wqkv[cb, :, :])
            b_sb = cpool.tile([F, 3], f32)
            nc.scalar.dma_start(b_sb[:], bqkv[:])
            eye_sb = cpool.tile([128, 128], bf16)
            nc.scalar.dma_start(eye_sb[:], eye[:])
            tri_sb = cpool.tile([KB, KB], bf16)
            nc.scalar.dma_start(tri_sb[:], tri[:])
            wp_sb = cpool.tile([F, C], bf16)
            nc.scalar.dma_start(wp_sb[:], wp[:])
            sel_sb = cpool.tile([33, 128], bf16)
            nc.scalar.dma_start(sel_sb[:], sel[:])
            for cb in range(8):
                nc.sync.dma_start(x_sb[:, cb, TCH:T], xt[cb, :, TCH:T])
            for cb in range(8):
                nc.sync.dma_start(x_sb[:, cb, T:], xt[cb, :, T:])

            # ---- resident sequence tensors (per 512-token chunk tiles) ----
            qt_t = [seq.tile([F, TCH], bf16, tag=f"qt{t}", name=f"qt{t}")
                    for t in range(NCH)]
            kt_t = [seq.tile([F, TCH], bf16, tag=f"kt{t}", name=f"kt{t}")
                    for t in range(NCH)]
            # v65[:, i, 0:65] = [V_headA | 1], v65[:, i, 65:130] = [V_headB | 1]
            v65 = seq.tile([128, BT // KB, 130], bf16)
            nc.vector.memset(v65[:, :, 64::65], 1.0)
            yt_t = [seq.tile([F, TCH], bf16, tag=f"yt{t}", name=f"yt{t}")
                    for t in range(NCH)]

            # ---- pre-zeroed diagonal P tiles (paired heads: [128,1024]) ----
            # halves: h0 cols [0:512), h1 cols [512:1024). For a diagonal
            # block with offset r, cols [0:128r) of each half are always
            # zero; zero them once, never rewrite.
            p_diag = {}
            for r in (1, 2, 3):
                for j in (0, 1):
                    pt = seq.tile([128, 2 * TCH], bf16, name=f"pdiag{r}_{j}")
                    pt3 = pt[:].rearrange("p (a q) -> p a q", a=2)
                    nc.vector.memset(pt3[:, :, 0:128 * r], 0.0)
                    p_diag[(r, j)] = pt

            # per-(b,bq,h) denominator rows [1, 512] f32 (partition offsets
            # must be multiples of 32, so no packing two heads in one tile)
            # and per-(b,bq) bf16 reciprocal rows at partitions 0 / 32 for
            # the sel33 broadcast matmul.
            den1 = {}
            rb2 = {}
            for b in range(B):
                for bq in range(NQC):
                    rb2[(b, bq)] = seq.tile([33, TCH], bf16,
                                            name=f"rb2{b}{bq}")
                    nc.vector.memset(rb2[(b, bq)][:], 0.0)
                    for h in range(HPC):
                        den1[(b, bq, h)] = seq.tile([1, TCH], f32,
                                                    name=f"den{b}{bq}{h}")

            def qkv_chunk_gen(t):
                """qkv projection + V transpose for one 512-token chunk.
                Part-outer: one PSUM accumulator live at a time."""
                vt_tmp = None
                for part in range(3):
                    ps = psBig.tile([128, TCH], f32, tag="big",
                                    name=f"pqkv{t}_{part}")
                    for cb in range(8):
                        nc.tensor.matmul(
                            ps[:], w_sb[:, cb, part * F:(part + 1) * F],
                            x_sb[:, cb, t * TCH:(t + 1) * TCH],
                            start=(cb == 0), stop=(cb == 7))
                        if cb == 3:
                            yield
                    if part == 0:
                        nc.vector.tensor_scalar_add(qt_t[t][:], ps[:],
                                                    b_sb[:, 0:1])
                    elif part == 1:
                        nc.vector.tensor_scalar_add(kt_t[t][:], ps[:],
                                                    b_sb[:, 1:2])
                    else:
                        vt_tmp = work.tile([128, TCH], bf16, tag="vt",
                                           name=f"vt{t}")
                        nc.vector.tensor_scalar_add(vt_tmp[:], ps[:],
                                                    b_sb[:, 2:3])
                    yield
                ptr = psS.tile([128, TCH], bf16, tag="s", name=f"ptr{t}")
                for i in range(4):
                    nc.tensor.transpose(ptr[:, i * 128:(i + 1) * 128],
                                        vt_tmp[:, i * 128:(i + 1) * 128],
                                        eye_sb[:])
                    if i == 1:
                        yield
                ptr3 = ptr[:].rearrange("p (a k) -> p a k", k=128)
                t4 = t * 4
                nc.vector.tensor_copy(v65[:, t4:t4 + 4, 0:64],
                                      ptr3[:, :, 0:64])
                nc.vector.tensor_copy(v65[:, t4:t4 + 4, 65:129],
                                      ptr3[:, :, 64:128])
                yield

            def norm_proj_gen(b, bq):
                """Per-chunk softmax normalization + projection."""
                qchunk = b * NQC + bq
                bc_ps = psBig.tile([128, TCH], f32, tag="big",
                                   name=f"bc{b}{bq}")
                nc.tensor.matmul(bc_ps[:], sel_sb[:], rb2[(b, bq)][:],
                                 start=True, stop=True)
                nc.vector.tensor_mul(yt_t[qchunk][:], yt_t[qchunk][:],
                                     bc_ps[:])
                yield
                for ic in range(4):
                    tt = qchunk * 4 + ic
                    for cc in range(2):
                        pj = psBig.tile([128, TCH], f32, tag="big",
                                        name=f"pj{tt}_{cc}")
                        nc.tensor.matmul(
                            pj[:],
                            yt_t[qchunk][:, ic * 128:(ic + 1) * 128],
                            wp_sb[:, cc * TCH:(cc + 1) * TCH],
                            start=True, stop=True)
                        ost = work.tile([128, TCH], bf16, tag="ost",
                                        name=f"ost{tt}_{cc}")
                        # batch-0 chunks retire while Scalar has slack;
                        # batch-1 chunks retire while EXP saturates Scalar,
                        # so their copies go to Vector.
                        if b == 0:
                            nc.scalar.copy(ost[:], pj[:])
                        else:
                            nc.vector.tensor_copy(ost[:], pj[:])
                        nc.sync.dma_start(
                            out[tt * 128:(tt + 1) * 128,
                                cc * TCH:(cc + 1) * TCH], ost[:])
                        yield

            class Filler:
                def __init__(self):
                    self.gens = []

                def add(self, g):
                    self.gens.append(g)

                def step(self):
                    while self.gens:
                        try:
                            next(self.gens[0])
                            return
                        except StopIteration:
                            self.gens.pop(0)

                def drain(self):
                    while self.gens:
                        for _ in self.gens.pop(0):
                            pass

            def attn_pair(b, bq, bk, use_idx):
                """S for both heads into one [128,1024] psum tile + one exp.
                Returns the P tile (halves = heads)."""
                qchunk = b * NQC + bq
                kchunk = b * NQC + bk // 4
                kcol = (bk % 4) * 128
                s_ps = psS.tile([128, 2 * TCH], f32, tag="s",
                                name=f"s{b}{bq}{bk}")
                r = bk - 4 * bq
                # masked q-columns [0:128r) skipped entirely (bf16 runs at
                # full rate for any moving width)
                trim = 128 * r if r > 0 else 0
                for h in range(HPC):
                    hs = h * 64
                    nc.tensor.matmul(
                        s_ps[:, h * TCH + trim:(h + 1) * TCH],
                        kt_t[kchunk][hs:hs + 64, kcol:kcol + 128],
                        qt_t[qchunk][hs:hs + 64, trim:],
                        start=True, stop=True)
                if r < 0:
                    p_t = work.tile([128, 2 * TCH], bf16, tag="p", bufs=4,
                                    name=f"p{b}{bq}{bk}")
                    nc.scalar.activation(p_t[:], s_ps[:], Exp)
                    return p_t
                if r == 0:
                    p_t = work.tile([128, 2 * TCH], bf16, tag="p", bufs=4,
                                    name=f"p{b}{bq}{bk}")
                    nc.scalar.activation(p_t[:], s_ps[:], Exp)
                else:
                    p_t = p_diag[(r, use_idx % 2)]
                    s3 = s_ps[:].rearrange("p (a q) -> p a q", a=2)
                    p3 = p_t[:].rearrange("p (a q) -> p a q", a=2)
                    nc.scalar.activation(p3[:, :, 128 * r:],
                                         s3[:, :, 128 * r:], Exp)
                for h in range(HPC):
                    c0 = h * TCH + 128 * r
                    nc.vector.tensor_mul(p_t[:, c0:c0 + 128],
                                         p_t[:, c0:c0 + 128], tri_sb[:])
                return p_t

            def attention_chunk(b, bq, fl, use_idx):
                qchunk = b * NQC + bq
                nblk = 4 * bq + 4
                yt_ps = [psY.tile([65, TCH], f32, tag="yt",
                                  name=f"ytps{b}{bq}{h}")
                         for h in range(HPC)]
                pend = None

                def emit_pv(bk, p_t, stop):
                    vti = b * (T // KB) + bk
                    r = bk - 4 * bq
                    trim = 128 * r if r > 0 else 0
                    for h in range(HPC):
                        nc.tensor.matmul(
                            yt_ps[h][:, trim:], v65[:, vti, 65 * h:65 * h + 65],
                            p_t[:, h * TCH + trim:(h + 1) * TCH],
                            start=(bk == 0), stop=stop)

                for bk in range(nblk):
                    p_t = attn_pair(b, bq, bk, use_idx)
                    if pend is not None:
                        emit_pv(pend[0], pend[1], stop=False)
                    pend = (bk, p_t)
                    fl.step()
                    fl.step()
                emit_pv(pend[0], pend[1], stop=True)
                for h in range(HPC):
                    hs = h * 64
                    nc.vector.tensor_copy(yt_t[qchunk][hs:hs + 64, :],
                                          yt_ps[h][0:64, :])
                    nc.vector.tensor_copy(den1[(b, bq, h)][:],
                                          yt_ps[h][64:65, :])
                fl.step()
                for h in range(HPC):
                    rec2 = work.tile([1, TCH], f32, tag="rec2",
                                     name=f"rec2{b}{bq}{h}")
                    nc.vector.reciprocal_approx_fast(
                        out=rec2[:], in_=den1[(b, bq, h)][:])
                    nc.vector.tensor_copy(rb2[(b, bq)][32 * h:32 * h + 1, :],
                                          rec2[:])
                fl.step()

            def chain(*gens):
                for g in gens:
                    yield from g

            # ---- schedule ----
            # Only qkv(0) runs to completion before attention starts;
            # qkv(1..5) fill batch-0 attention, qkv(6,7) fill batch-1.
            # Batch-1 runs bq1,bq2,bq3,bq0 so the smallest chunk is the
            # serial tail.
            for _ in qkv_chunk_gen(0):
                pass
            fl = Filler()
            fl.add(chain(*[qkv_chunk_gen(t) for t in range(1, NCH)]))
            order = [(0, 0), (0, 1), (0, 2), (0, 3),
                     (1, 1), (1, 2), (1, 3), (1, 0)]
            pending_np = []
            for idx, (b, bq) in enumerate(order):
                if idx == len(order) - 1:
                    # last chunk: make all pending norm+proj work available
                    # as filler so only the final chunk's chain remains at
                    # the tail
                    while pending_np:
                        fl.add(pending_np.pop(0))
                attention_chunk(b, bq, fl, idx)
                if pending_np:
                    fl.add(pending_np.pop(0))
                pending_np.append(norm_proj_gen(b, bq))
            for g in pending_np:
                fl.add(g)
            fl.drain()
    nc.compile()
    return nc


def _get_nc():
    if "nc" not in _COMPILED:
        _COMPILED["nc"] = _build()
    return _COMPILED["nc"]


def _prep_in_maps(x, w_attn, b_attn, w_proj):
    import ml_dtypes
    bf16 = ml_dtypes.bfloat16

    x = np.asarray(x, np.float32)
    w_attn = np.asarray(w_attn, np.float32)
    b_attn = np.asarray(b_attn, np.float32)
    w_proj = np.asarray(w_proj, np.float32)

    scale = np.float32(1.0 / np.sqrt(D))
    xt = np.ascontiguousarray(
        x.reshape(BT, C).T.reshape(C // 128, 128, BT)).astype(bf16)
    # tri[kv, j] = 1 when j >= kv (upper triangular incl diagonal)
    tri = np.triu(np.ones((KB, KB), np.float32)).astype(bf16)
    eye = np.eye(128, dtype=np.float32).astype(bf16)
    sel = np.zeros((33, 128), np.float32)
    sel[0, 0:64] = 1.0
    sel[32, 64:128] = 1.0
    sel = sel.astype(bf16)

    in_maps = []
    for c in range(N_CORES):
        cols = slice(c * F, (c + 1) * F)
        wq = w_attn[:, cols] * scale
        wk = w_attn[:, C + c * F:C + (c + 1) * F]
        wv = w_attn[:, 2 * C + c * F:2 * C + (c + 1) * F]
        wqkv = np.ascontiguousarray(
            np.concatenate([wq, wk, wv], axis=1).reshape(
                C // 128, 128, 3 * F)).astype(bf16)
        bq = b_attn[c * F:(c + 1) * F] * scale
        bk = b_attn[C + c * F:C + (c + 1) * F]
        bv = b_attn[2 * C + c * F:2 * C + (c + 1) * F]
        bqkv = np.ascontiguousarray(np.stack([bq, bk, bv], axis=1))
        wpc = np.ascontiguousarray(w_proj[c * F:(c + 1) * F, :]).astype(bf16)
        in_maps.append({
            "xt": xt, "wqkv": wqkv, "bqkv": bqkv, "wp": wpc,
            "tri": tri, "eye": eye, "sel": sel,
        })
    return in_maps


def _run(inputs, trace=False):
    from concourse.bass_utils import run_bass_kernel_spmd

    nc = _get_nc()
    in_maps = _prep_in_maps(inputs["x"], inputs["w_attn"], inputs["b_attn"],
                            inputs["w_proj"])
    res = run_bass_kernel_spmd(nc, in_maps, list(range(N_CORES)), trace=trace)
    b_proj = np.asarray(inputs["b_proj"], np.float32)
    acc = np.zeros((BT, C), np.float64)
    for c in range(N_CORES):
        acc += res.results[c]["out"].astype(np.float64)
    y = (acc + b_proj).astype(np.float32).reshape(B, T, C)
    return y, res


def kernel(**inputs):
    y, _ = _run(inputs, trace=False)
    return y


# revision 32
# speedup vs baseline: 1.0523x; 1.0523x over previous
"""Causal self-attention (B=2, T=2048, C=1024, H=16) on 8 trn2 NeuronCores.

Sharding: tensor-parallel over heads. Each core owns 2 heads (128 features):
  - qkv projection of the full sequence onto its 384 w_attn columns
  - causal attention for its 2 heads (both batches)
  - partial c_proj: y_local [4096,128] @ w_proj[rows] -> [4096,1024] partial
The 8 partial outputs are summed on the host (the "all-reduce after c_proj"),
plus b_proj.

Everything on-device runs in bf16 with fp32 PSUM accumulation (measured
end-to-end absmax-relative error vs the fp32 reference ~3e-3 in numpy
simulation). bf16 halves DMA traffic and DVE element-wise cost, and runs
matmuls at full rate for any moving width (so causal diagonal blocks trim
to 128-column granularity).

Layout: x is transposed on the host (x^T [1024, 4096], bf16) so the qkv
projection consumes it directly as the moving operand; Q^T/K^T come out in
[feature, token] layout feeding the S = K-stationary attention matmul, with
softmax denominators obtained from an extra ones-column in V. Normalization
uses reciprocal_approx_fast on the PSUM denominator row + gpsimd partition
broadcast (no DRAM round trip).
"""

import sys

sys.path.insert(0, "/opt/trn_rl_repo")

import numpy as np

N_CORES = 8
B, T, C = 2, 2048, 1024
H, D = 16, 64
HPC = H // N_CORES            # heads per core
F = HPC * D                   # local feature width = 128
BT = B * T                    # 4096 tokens
TCH = 512                     # token chunk (moving-operand width)
NCH = BT // TCH               # 8 token chunks
KB = 128                      # kv block size
NQC = T // TCH                # 4 query chunks per batch

_COMPILED = {}


def _build():
    import concourse.bass as bass
    import concourse.mybir as mybir
    import concourse.tile as tile
    from concourse import bacc

    f32, bf16 = mybir.dt.float32, mybir.dt.bfloat16
    Exp = mybir.ActivationFunctionType.Exp

    nc = bacc.Bacc("TRN2", target_bir_lowering=False, debug=False,
                   num_devices=N_CORES)

    xt = nc.dram_tensor("xt", [C // 128, 128, BT], bf16, kind="ExternalInput")
    wqkv = nc.dram_tensor("wqkv", [C // 128, 128, 3 * F], bf16,
                          kind="ExternalInput")
    bqkv = nc.dram_tensor("bqkv", [F, 3], f32, kind="ExternalInput")
    wp = nc.dram_tensor("wp", [F, C], bf16, kind="ExternalInput")
    tri = nc.dram_tensor("tri", [KB, KB], bf16, kind="ExternalInput")
    eye = nc.dram_tensor("eye", [128, 128], bf16, kind="ExternalInput")
    # sel33[0, p] = 1 for p < 64; sel33[32, p] = 1 for p >= 64; else 0.
    # One matmul (sel33^T @ rb2) broadcasts per-head reciprocal rows to a
    # [128, 512] tile: partitions 0-63 get rb2 row 0, 64-127 get row 32.
    sel = nc.dram_tensor("sel", [33, 128], bf16, kind="ExternalInput")
    out = nc.dram_tensor("out", [BT, C], bf16, kind="ExternalOutput")

    with tile.TileContext(nc) as tc, \
         nc.allow_low_precision(reason="bf16 matmul pipeline, fp32 psum"):
        with tc.tile_pool(name="const", bufs=1) as cpool, \
             tc.tile_pool(name="seq", bufs=1) as seq, \
             tc.tile_pool(name="work", bufs=4) as work, \
             tc.tile_pool(name="psBig", bufs=2, space="PSUM") as psBig, \
             tc.tile_pool(name="psS", bufs=2, space="PSUM") as psS, \
             tc.tile_pool(name="psY", bufs=2, space="PSUM") as psY:

            # ---- resident x (bf16, 64KB/partition) + constants ----
            # issue order matters: interleave x chunk 0 and w per-cb so the
            # first qkv matmuls can start as early as possible; the rest of
            # x follows as one big per-cb transfer. Small constants go on
            # the scalar (Activation) hwdge queue, which is idle at start.
            x_sb = cpool.tile([128, C // 128, BT], bf16)
            w_sb = cpool.tile([128, C // 128, 3 * F], bf16)
            for cb in range(8):
                nc.sync.dma_start(x_sb[:, cb, 0:TCH], xt[cb, :, 0:TCH])
                nc.sync.dma_start(w_sb[:, cb, :], wqkv[cb, :, :])
            b_sb = cpool.tile([F, 3], f32)
            nc.scalar.dma_start(b_sb[:], bqkv[:])
            eye_sb = cpool.tile([128, 128], bf16)
            nc.scalar.dma_start(eye_sb[:], eye[:])
            tri_sb = cpool.tile([KB, KB], bf16)
            nc.scalar.dma_start(tri_sb[:], tri[:])
            wp_sb = cpool.tile([F, C], bf16)
            nc.scalar.dma_start(wp_sb[:], wp[:])
            sel_sb = cpool.tile([33, 128], bf16)
            nc.scalar.dma_start(sel_sb[:], sel[:])
            for cb in range(8):
                nc.sync.dma_start(x_sb[:, cb, TCH:T], xt[cb, :, TCH:T])

            # ---- resident sequence tensors (per 512-token chunk tiles) ----
            qt_t = [seq.tile([F, TCH], bf16, tag=f"qt{t}", name=f"qt{t}")
                    for t in range(NCH)]
            kt_t = [seq.tile([F, TCH], bf16, tag=f"kt{t}", name=f"kt{t}")
                    for t in range(NCH)]
            # v65[:, i, 0:65] = [V_headA | 1], v65[:, i, 65:130] = [V_headB | 1]
            v65 = seq.tile([128, BT // KB, 130], bf16)
            nc.vector.memset(v65[:, :, 64::65], 1.0)
            yt_t = [seq.tile([F, TCH], bf16, tag=f"yt{t}", name=f"yt{t}")
                    for t in range(NCH)]

            # ---- pre-zeroed diagonal P tiles (paired heads: [128,1024]) ----
            # halves: h0 cols [0:512), h1 cols [512:1024). For a diagonal
            # block with offset r, cols [0:128r) of each half are always
            # zero; zero them once, never rewrite.
            p_diag = {}
            for r in (1, 2, 3):
                for j in (0, 1):
                    pt = seq.tile([128, 2 * TCH], bf16, name=f"pdiag{r}_{j}")
                    pt3 = pt[:].rearrange("p (a q) -> p a q", a=2)
                    nc.vector.memset(pt3[:, :, 0:128 * r], 0.0)
                    p_diag[(r, j)] = pt

            # per-(b,bq,h) denominator rows [1, 512] f32 (partition offsets
            # must be multiples of 32, so no packing two heads in one tile)
            # and per-(b,bq) bf16 reciprocal rows at partitions 0 / 32 for
            # the sel33 broadcast matmul.
            den1 = {}
            rb2 = {}
            for b in range(B):
                for bq in range(NQC):
                    rb2[(b, bq)] = seq.tile([33, TCH], bf16,
                                            name=f"rb2{b}{bq}")
                    nc.vector.memset(rb2[(b, bq)][:], 0.0)
                    for h in range(HPC):
                        den1[(b, bq, h)] = seq.tile([1, TCH], f32,
                                                    name=f"den{b}{bq}{h}")

            def qkv_chunk_gen(t):
                """qkv projection + V transpose for one 512-token chunk.
                Part-outer: one PSUM accumulator live at a time."""
                vt_tmp = None
                for part in range(3):
                    ps = psBig.tile([128, TCH], f32, tag="big",
                                    name=f"pqkv{t}_{part}")
                    for cb in range(8):
                        nc.tensor.matmul(
                            ps[:], w_sb[:, cb, part * F:(part + 1) * F],
                            x_sb[:, cb, t * TCH:(t + 1) * TCH],
                            start=(cb == 0), stop=(cb == 7))
                        if cb == 3:
                            yield
                    if part == 0:
                        nc.vector.tensor_scalar_add(qt_t[t][:], ps[:],
                                                    b_sb[:, 0:1])
                    elif part == 1:
                        nc.vector.tensor_scalar_add(kt_t[t][:], ps[:],
                                                    b_sb[:, 1:2])
                    else:
                        vt_tmp = work.tile([128, TCH], bf16, tag="vt",
                                           name=f"vt{t}")
                        nc.vector.tensor_scalar_add(vt_tmp[:], ps[:],
                                                    b_sb[:, 2:3])
                    yield
                ptr = psS.tile([128, TCH], bf16, tag="s", name=f"ptr{t}")
                for i in range(4):
                    nc.tensor.transpose(ptr[:, i * 128:(i + 1) * 128],
                                        vt_tmp[:, i * 128:(i + 1) * 128],
                                        eye_sb[:])
                    if i == 1:
                        yield
                ptr3 = ptr[:].rearrange("p (a k) -> p a k", k=128)
                t4 = t * 4
                nc.vector.tensor_copy(v65[:, t4:t4 + 4, 0:64],
                                      ptr3[:, :, 0:64])
                nc.vector.tensor_copy(v65[:, t4:t4 + 4, 65:129],
                                      ptr3[:, :, 64:128])
                yield

            def norm_proj_gen(b, bq):
                """Per-chunk softmax normalization + projection."""
                qchunk = b * NQC + bq
                bc_ps = psBig.tile([128, TCH], f32, tag="big",
                                   name=f"bc{b}{bq}")
                nc.tensor.matmul(bc_ps[:], sel_sb[:], rb2[(b, bq)][:],
                                 start=True, stop=True)
                nc.vector.tensor_mul(yt_t[qchunk][:], yt_t[qchunk][:],
                                     bc_ps[:])
                yield
                for ic in range(4):
                    tt = qchunk * 4 + ic
                    for cc in range(2):
                        pj = psBig.tile([128, TCH], f32, tag="big",
                                        name=f"pj{tt}_{cc}")
                        nc.tensor.matmul(
                            pj[:],
                            yt_t[qchunk][:, ic * 128:(ic + 1) * 128],
                            wp_sb[:, cc * TCH:(cc + 1) * TCH],
                            start=True, stop=True)
                        ost = work.tile([128, TCH], bf16, tag="ost",
                                        name=f"ost{tt}_{cc}")
                        # batch-0 chunks retire while Scalar has slack;
                        # batch-1 chunks retire while EXP saturates Scalar,
                        # so their copies go to Vector.
                        if b == 0:
                            nc.scalar.copy(ost[:], pj[:])
                        else:
                            nc.vector.tensor_copy(ost[:], pj[:])
                        nc.sync.dma_start(
                            out[tt * 128:(tt + 1) * 128,
                                cc * TCH:(cc + 1) * TCH], ost[:])
                        yield

            class Filler:
                """Two-queue filler: `pri` (qkv, must stay ahead of
                attention) gets 2 of every 3 steps; `sec` (norm+proj)
                retires continuously instead of piling up at the tail."""

                def __init__(self):
                    self.pri = []
                    self.sec = []
                    self.tick = 0

                def add_pri(self, g):
                    self.pri.append(g)

                def add(self, g):
                    self.sec.append(g)

                def _pull(self, q):
                    while q:
                        try:
                            next(q[0])
                            return True
                        except StopIteration:
                            q.pop(0)
                    return False

                def step(self):
                    self.tick += 1
                    first = self.sec if self.tick % 3 == 0 else self.pri
                    other = self.pri if first is self.sec else self.sec
                    if not self._pull(first):
                        self._pull(other)

                def drain(self):
                    while self._pull(self.pri) or self._pull(self.sec):
                        pass

            def attn_pair(b, bq, bk, use_idx):
                """S for both heads into one [128,1024] psum tile + one exp.
                Returns the P tile (halves = heads)."""
                qchunk = b * NQC + bq
                kchunk = b * NQC + bk // 4
                kcol = (bk % 4) * 128
                s_ps = psS.tile([128, 2 * TCH], f32, tag="s",
                                name=f"s{b}{bq}{bk}")
                r = bk - 4 * bq
                # masked q-columns [0:128r) skipped entirely (bf16 runs at
                # full rate for any moving width)
                trim = 128 * r if r > 0 else 0
                for h in range(HPC):
                    hs = h * 64
                    nc.tensor.matmul(
                        s_ps[:, h * TCH + trim:(h + 1) * TCH],
                        kt_t[kchunk][hs:hs + 64, kcol:kcol + 128],
                        qt_t[qchunk][hs:hs + 64, trim:],
                        start=True, stop=True)
                if r < 0:
                    p_t = work.tile([128, 2 * TCH], bf16, tag="p", bufs=4,
                                    name=f"p{b}{bq}{bk}")
                    nc.scalar.activation(p_t[:], s_ps[:], Exp)
                    return p_t
                if r == 0:
                    p_t = work.tile([128, 2 * TCH], bf16, tag="p", bufs=4,
                                    name=f"p{b}{bq}{bk}")
                    nc.scalar.activation(p_t[:], s_ps[:], Exp)
                else:
                    p_t = p_diag[(r, use_idx % 2)]
                    s3 = s_ps[:].rearrange("p (a q) -> p a q", a=2)
                    p3 = p_t[:].rearrange("p (a q) -> p a q", a=2)
                    nc.scalar.activation(p3[:, :, 128 * r:],
                                         s3[:, :, 128 * r:], Exp)
                for h in range(HPC):
                    c0 = h * TCH + 128 * r
                    nc.vector.tensor_mul(p_t[:, c0:c0 + 128],
                                         p_t[:, c0:c0 + 128], tri_sb[:])
                return p_t

            def attention_chunk(b, bq, fl, use_idx):
                qchunk = b * NQC + bq
                nblk = 4 * bq + 4
                yt_ps = [psY.tile([65, TCH], f32, tag="yt",
                                  name=f"ytps{b}{bq}{h}")
                         for h in range(HPC)]
                pend = None

                def emit_pv(bk, p_t, stop):
                    vti = b * (T // KB) + bk
                    r = bk - 4 * bq
                    trim = 128 * r if r > 0 else 0
                    for h in range(HPC):
                        nc.tensor.matmul(
                            yt_ps[h][:, trim:], v65[:, vti, 65 * h:65 * h + 65],
                            p_t[:, h * TCH + trim:(h + 1) * TCH],
                            start=(bk == 0), stop=stop)

                for bk in range(nblk):
                    p_t = attn_pair(b, bq, bk, use_idx)
                    if pend is not None:
                        emit_pv(pend[0], pend[1], stop=False)
                    pend = (bk, p_t)
                    fl.step()
                    fl.step()
                emit_pv(pend[0], pend[1], stop=True)
                for h in range(HPC):
                    hs = h * 64
                    nc.vector.tensor_copy(yt_t[qchunk][hs:hs + 64, :],
                                          yt_ps[h][0:64, :])
                    nc.vector.tensor_copy(den1[(b, bq, h)][:],
                                          yt_ps[h][64:65, :])
                fl.step()
                for h in range(HPC):
                    rec2 = work.tile([1, TCH], f32, tag="rec2",
                                     name=f"rec2{b}{bq}{h}")
                    nc.vector.reciprocal_approx_fast(
                        out=rec2[:], in_=den1[(b, bq, h)][:])
                    nc.vector.tensor_copy(rb2[(b, bq)][32 * h:32 * h + 1, :],
                                          rec2[:])
                fl.step()

            def chain(*gens):
                for g in gens:
                    yield from g

            # ---- schedule ----
            # Only qkv(0) runs to completion before attention starts;
            # qkv(1..5) fill batch-0 attention, qkv(6,7) fill batch-1.
            # Batch-1 runs bq1,bq2,bq3,bq0 so the smallest chunk is the
            # serial tail.
            for _ in qkv_chunk_gen(0):
                pass
            fl = Filler()

            def qkv_rest():
                # defer batch-1 x loads until batch-0 x has the HBM
                # bandwidth to itself
                yield from qkv_chunk_gen(1)
                for cb in range(8):
                    nc.sync.dma_start(x_sb[:, cb, T:], xt[cb, :, T:])
                for t in range(2, NCH):
                    yield from qkv_chunk_gen(t)

            fl.add_pri(qkv_rest())
            order = [(0, 0), (0, 1), (0, 2), (0, 3),
                     (1, 1), (1, 2), (1, 3), (1, 0)]
            pending_np = []
            for idx, (b, bq) in enumerate(order):
                if idx == len(order) - 1:
                    # last chunk: make all pending norm+proj work available
                    # as filler so only the final chunk's chain remains at
                    # the tail
                    while pending_np:
                        fl.add(pending_np.pop(0))
                attention_chunk(b, bq, fl, idx)
                if pending_np:
                    fl.add(pending_np.pop(0))
                pending_np.append(norm_proj_gen(b, bq))
            for g in pending_np:
                fl.add(g)
            fl.drain()
    nc.compile()
    return nc


def _get_nc():
    if "nc" not in _COMPILED:
        _COMPILED["nc"] = _build()
    return _COMPILED["nc"]


def _prep_in_maps(x, w_attn, b_attn, w_proj):
    import ml_dtypes
    bf16 = ml_dtypes.bfloat16

    x = np.asarray(x, np.float32)
    w_attn = np.asarray(w_attn, np.float32)
    b_attn = np.asarray(b_attn, np.float32)
    w_proj = np.asarray(w_proj, np.float32)

    scale = np.float32(1.0 / np.sqrt(D))
    xt = np.ascontiguousarray(
        x.reshape(BT, C).T.reshape(C // 128, 128, BT)).astype(bf16)
    # tri[kv, j] = 1 when j >= kv (upper triangular incl diagonal)
    tri = np.triu(np.ones((KB, KB), np.float32)).astype(bf16)
    eye = np.eye(128, dtype=np.float32).astype(bf16)
    sel = np.zeros((33, 128), np.float32)
    sel[0, 0:64] = 1.0
    sel[32, 64:128] = 1.0
    sel = sel.astype(bf16)

    in_maps = []
    for c in range(N_CORES):
        cols = slice(c * F, (c + 1) * F)
        wq = w_attn[:, cols] * scale
        wk = w_attn[:, C + c * F:C + (c + 1) * F]
        wv = w_attn[:, 2 * C + c * F:2 * C + (c + 1) * F]
        wqkv = np.ascontiguousarray(
            np.concatenate([wq, wk, wv], axis=1).reshape(
                C // 128, 128, 3 * F)).astype(bf16)
        bq = b_attn[c * F:(c + 1) * F] * scale
        bk = b_attn[C + c * F:C + (c + 1) * F]
        bv = b_attn[2 * C + c * F:2 * C + (c + 1) * F]
        bqkv = np.ascontiguousarray(np.stack([bq, bk, bv], axis=1))
        wpc = np.ascontiguousarray(w_proj[c * F:(c + 1) * F, :]).astype(bf16)
        in_maps.append({
            "xt": xt, "wqkv": wqkv, "bqkv": bqkv, "wp": wpc,
            "tri": tri, "eye": eye, "sel": sel,
        })
    return in_maps


def _run(inputs, trace=False):
    from concourse.bass_utils import run_bass_kernel_spmd

    nc = _get_nc()
    in_maps = _prep_in_maps(inputs["x"], inputs["w_attn"], inputs["b_attn"],
                            inputs["w_proj"])
    res = run_bass_kernel_spmd(nc, in_maps, list(range(N_CORES)), trace=trace)
    b_proj = np.asarray(inputs["b_proj"], np.float32)
    acc = np.zeros((BT, C), np.float64)
    for c in range(N_CORES):
        acc += res.results[c]["out"].astype(np.float64)
    y = (acc + b_proj).astype(np.float32).reshape(B, T, C)
    return y, res


def kernel(**inputs):
    y, _ = _run(inputs, trace=False)
    return y
